# revision 5
# baseline (speedup 1.0000x reference)
"""Trainium2 Bass kernel for nn_CalculateHLayer (GNN message passing).

Computes, for adj [4096, 4096, 2] f32 and h [4096, 150] f32:
    A     = adj.sum(axis=2)          # [L, L]
    h_in  = A.T @ h                  # [L, D]
    h_out = A @ h                    # [L, D]
returning (h_in, h_out) as float32, matching the reference.

Distribution: adj is sharded row-wise (dim 0) across 8 NeuronCores, h is
replicated.  Each core computes its local rows of h_out directly and a
partial h_in (contraction over its local i rows); the 8 h_in partials are
summed on the host.

Per-core dataflow (Tile framework):
  - DMA adj row-stripe chunks [128 i, 512 j, 2 e] f32 into SBUF.
  - DVE edge-sum adj[...,0] + adj[...,1] -> A tile bf16.
  - h_in:  matmul(psum, lhsT=A[i,j] tile, rhs=h_local[i,d]) accumulating
           over the 4 local i tiles, one PSUM bank per j tile, evacuated
           to DRAM each j tile.
  - h_out: PE-transpose each 128x128 A tile (identity matmul), then
           matmul(psum, lhsT=A_T[j,i], rhs=h[j,d]) accumulating over all
           32 j tiles in 4 persistent PSUM banks (one per local i tile).
Matmuls run in bf16 (inputs are O(1) magnitudes; PSUM accumulates fp32).
"""

import sys

for _p in ("/opt/trn_rl_repo",):
    if _p not in sys.path:
        sys.path.append(_p)

from contextlib import ExitStack

import numpy as np

import concourse.bass as bass
import concourse.mybir as mybir
import concourse.tile as tile
from concourse import bacc
from concourse.bass_utils import run_bass_kernel_spmd
from concourse.masks import make_identity

L = 4096          # number of nodes
D = 150           # feature dim
NCORES = 8
R = L // NCORES   # rows of adj per core (512)
P = 128           # SBUF partitions
IT = R // P       # i tiles per core (4)
JT = L // P       # j tiles (32)
JC = 512          # j-chunk width per adj DMA
NJC = L // JC     # 8 chunks
JPC = JC // P     # j tiles per chunk (4)

F32 = mybir.dt.float32
BF16 = mybir.dt.bfloat16

_NC_CACHE = None


def _build():
    global _NC_CACHE
    if _NC_CACHE is not None:
        return _NC_CACHE

    nc = bacc.Bacc()
    adj = nc.declare_dram_parameter("adj", [R, L, 2], F32, isOutput=False)
    h = nc.declare_dram_parameter("h", [L, D], F32, isOutput=False)
    hloc = nc.declare_dram_parameter("hloc", [R, D], F32, isOutput=False)
    hin = nc.declare_dram_parameter("hin", [L, D], F32, isOutput=True)
    hout = nc.declare_dram_parameter("hout", [R, D], F32, isOutput=True)

    with ExitStack() as ctx:
        tc = ctx.enter_context(tile.TileContext(nc))
        const = ctx.enter_context(tc.tile_pool(name="const", bufs=1))
        stage = ctx.enter_context(tc.tile_pool(name="stage", bufs=1))
        adjp = ctx.enter_context(tc.tile_pool(name="adjp", bufs=6))
        abfp = ctx.enter_context(tc.tile_pool(name="abfp", bufs=2))
        atp = ctx.enter_context(tc.tile_pool(name="atp", bufs=4))
        evp = ctx.enter_context(tc.tile_pool(name="evp", bufs=4))
        ps_hin = ctx.enter_context(tc.tile_pool(name="ps_hin", bufs=2, space="PSUM"))
        ps_tr = ctx.enter_context(tc.tile_pool(name="ps_tr", bufs=2, space="PSUM"))
        ps_hout = ctx.enter_context(tc.tile_pool(name="ps_hout", bufs=1, space="PSUM"))

        ident = const.tile([P, P], BF16)
        make_identity(nc, ident)

        # DRAM views tiled to 128 partitions (row = o*128 + p)
        h_t = h.rearrange("(o p) d -> p o d", p=P)          # [128, 32, 150]
        hloc_t = hloc.rearrange("(o p) d -> p o d", p=P)    # [128, 4, 150]
        hin_t = hin.rearrange("(o p) d -> p o d", p=P)
        hout_t = hout.rearrange("(o p) d -> p o d", p=P)
        adj_t = adj.rearrange("(io p) l e -> io p (l e)", p=P)  # [4, 128, 8192]

        # Preload h (replicated) and the core's local h rows; cast to bf16.
        hf = stage.tile([P, JT, D], F32)
        nc.sync.dma_start(hf, h_t)
        hbf = const.tile([P, JT, D], BF16)
        nc.any.tensor_copy(hbf, hf)

        hlf = stage.tile([P, IT, D], F32)
        nc.sync.dma_start(hlf, hloc_t)
        hlbf = const.tile([P, IT, D], BF16)
        nc.any.tensor_copy(hlbf, hlf)

        # Persistent PSUM accumulators for the core's 4 h_out row tiles.
        phout = [ps_hout.tile([P, D], F32, name=f"phout{it}") for it in range(IT)]

        for jc in range(NJC):
            a_bf = []
            for it in range(IT):
                adj_sb = adjp.tile([P, JC * 2], F32, tag="adj")
                nc.sync.dma_start(
                    adj_sb, adj_t[it, :, jc * JC * 2 : (jc + 1) * JC * 2]
                )
                ab = abfp.tile([P, JC], BF16, tag=f"abf{it}")
                av = adj_sb.rearrange("p (j e) -> p j e", e=2)
                nc.vector.tensor_add(ab, av[:, :, 0], av[:, :, 1])
                a_bf.append(ab)

            for j8 in range(JPC):
                jt = jc * JPC + j8
                jsl = bass.ts(j8, P)

                # h_in[j-tile] = sum_it A[it, j-tile].T @ h_local[it]
                pin = ps_hin.tile([P, D], F32, tag="phin")
                for it in range(IT):
                    nc.tensor.matmul(
                        pin,
                        lhsT=a_bf[it][:, jsl],
                        rhs=hlbf[:, it, :],
                        start=(it == 0),
                        stop=(it == IT - 1),
                    )
                ev = evp.tile([P, D], F32, tag="ev")
                nc.any.tensor_copy(ev, pin)
                nc.sync.dma_start(hin_t[:, jt, :], ev)

                # h_out[it] += A[it, j-tile] @ h[j-tile]
                for it in range(IT):
                    ptr = ps_tr.tile([P, P], BF16, tag="ptr")
                    nc.tensor.transpose(ptr, a_bf[it][:, jsl], ident)
                    at2 = atp.tile([P, P], BF16, tag="at")
                    nc.any.tensor_copy(at2, ptr)
                    nc.tensor.matmul(
                        phout[it],
                        lhsT=at2,
                        rhs=hbf[:, jt, :],
                        start=(jt == 0),
                        stop=(jt == JT - 1),
                    )

        for it in range(IT):
            ev = evp.tile([P, D], F32, tag="ev")
            nc.any.tensor_copy(ev, phout[it])
            nc.sync.dma_start(hout_t[:, it, :], ev)

    nc.compile()
    _NC_CACHE = nc
    return nc


def _run(adj, h, trace=False):
    nc = _build()
    in_maps = []
    for c in range(NCORES):
        sl = slice(c * R, (c + 1) * R)
        in_maps.append(
            {
                "adj": np.ascontiguousarray(adj[sl]),
                "h": h,
                "hloc": np.ascontiguousarray(h[sl]),
            }
        )
    return run_bass_kernel_spmd(nc, in_maps, list(range(NCORES)), trace=trace)


def kernel(**inputs):
    adj = np.ascontiguousarray(
        np.asarray(inputs["unpreprocessed_unweight_adj_matrix"], dtype=np.float32)
    )
    h = np.ascontiguousarray(np.asarray(inputs["h"], dtype=np.float32))

    res = _run(adj, h)
    outs = res.results
    h_in = outs[0]["hin"].astype(np.float32, copy=True)
    for c in range(1, NCORES):
        h_in += outs[c]["hin"]
    h_out = np.concatenate([outs[c]["hout"] for c in range(NCORES)], axis=0)
    return (h_in, h_out)
